# revision 2
# baseline (speedup 1.0000x reference)
"""Causal self-attention (RMSNorm + QKV + causal attention + out-proj) on 8 trn2
NeuronCores.

Sharding: core c handles batch b=c//2 and head-group g=c%2 (Megatron-style TP
over the 16 heads: 8 heads per group). Each core computes a partial output
y_part[b,g] = attn_out(heads of g) @ w_proj[:, g-cols].T ; the host sums the two
TP partials per batch (y shipped bf16, summed fp32 on host).

Restructured pipeline vs the first version: the attention steps (S/EXP/O/norm)
are interleaved with the remaining QKV/V projections and the out-proj c_blocks
so the ACT engine's EXP stream (the attention-phase bottleneck) overlaps the
PE-bound projection work instead of serializing after it. Other changes:
  - sumsq partition-reduce runs on bf16 squares (2.5x faster PE matmuls)
  - S tiles live in 2-bank [128,1024] PSUM pair tiles: one mask-prefill CAST
    and one EXP per pair (48 EXPs, no wasted columns)
  - input DMAs ride the two HWDGE queues (sync: xt+wv+mask, scalar: wqk+wproj)
    in need-order; y out is bf16
"""

import sys

sys.path.insert(0, "/opt/trn_rl_repo")

import numpy as np
import ml_dtypes

import concourse.bass as bass
import concourse.tile as tile
from concourse import mybir
from concourse.bass_utils import run_bass_kernel_spmd

BF16 = ml_dtypes.bfloat16

B, T, E, H = 4, 1024, 1024, 16
D = E // H  # 64
HL = 8  # heads per core (TP degree 2)
FL = HL * D  # 512 local head cols
EPS = 1e-5
N_CORES = 8
NEG = -2400.0  # additive mask; exp(0.125 * -2400) underflows fp32 to 0

AF = mybir.ActivationFunctionType
DT = mybir.dt


# ---------------------------------------------------------------------------
# Walrus in this toolchain rejects instructions whose tail Drain carries more
# than one semaphore wait; split the TileContext exit drain into chained
# single-wait drains.
def _patched_drain_and_barrier(self, tick_clock, wait_clock):
    nc = self.nc
    drain_inst = nc.sync.drain()
    wait_clock.add_sem_waits(
        drain_inst.ins, tile.ScopedClock({None: tick_clock.global_clock})
    )
    mi = drain_inst.ins
    si = mi.sync_info
    if si is not None and len(si.on_wait) > 1:
        waits = list(si.on_wait)
        mi.sync_info = mybir.SyncInfo(on_wait=waits[:1], on_update=list(si.on_update))
        for w in waits[1:]:
            extra = nc.sync.drain().ins
            extra.sync_info = mybir.SyncInfo(on_wait=[w], on_update=[])
    nc.all_engine_barrier()
    assert self.sems is not None
    popped = nc._tile_sem_poison_stack.pop()
    assert popped is self._sem_poison
    nc.clear_and_free_semaphores(list(self.sems.allocated().values()))
    nc.all_engine_barrier()


tile.TileContext._drain_and_barrier = _patched_drain_and_barrier

# The same 1-wait-per-instruction walrus limit applies to every engine's
# instructions. Legalize at BIR-serialization time: hoist all but the last
# wait of a multi-wait instruction onto fresh single-wait Drains inserted
# just before it on the same engine.
_orig_to_json_bytes = bass.Bass.to_json_bytes


def _legalized_to_json_bytes(self):
    import orjson

    j = orjson.loads(_orig_to_json_bytes(self))
    ctr = 0
    for fn in j["functions"]:
        for bb in fn["blocks"]:
            new_insts = []
            for ins in bb["instructions"]:
                si = ins.get("sync_info")
                waits = si.get("on_wait") if si else None
                if waits and len(waits) > 1:
                    for w in waits[:-1]:
                        ctr += 1
                        new_insts.append(
                            {
                                "debug": ins.get("debug"),
                                "engine": ins["engine"],
                                "ins": [],
                                "outs": [],
                                "name": f"I-wf{ctr}",
                                "opcode": "EventSemaphore",
                                "sync_info": {"on_update": [], "on_wait": [w]},
                            }
                        )
                    si["on_wait"] = [waits[-1]]
                new_insts.append(ins)
            bb["instructions"] = new_insts
    return orjson.dumps(j)


bass.Bass.to_json_bytes = _legalized_to_json_bytes


def build_program():
    nc = bass.Bass("TRN2", target_bir_lowering=False, debug=False)

    xt_d = nc.declare_dram_parameter("xt", [E, T], DT.bfloat16, isOutput=False)
    wqk_d = nc.declare_dram_parameter("wqk", [E, 2 * FL], DT.bfloat16, isOutput=False)
    wv_d = nc.declare_dram_parameter("wv", [E, FL], DT.bfloat16, isOutput=False)
    wproj_d = nc.declare_dram_parameter("wproj", [4, 128, E], DT.bfloat16, isOutput=False)
    mask_d = nc.declare_dram_parameter("mask", [128, 1280], DT.bfloat16, isOutput=False)
    y_d = nc.declare_dram_parameter("y", [T, E], DT.bfloat16, isOutput=True)

    NT = T // 128  # 8 tiles of 128

    with tile.TileContext(nc) as tc:
        with (
            tc.tile_pool(name="persist", bufs=1) as persist,
            tc.tile_pool(name="work", bufs=3) as work,
            tc.tile_pool(name="pP", bufs=8) as pP,
            tc.tile_pool(name="psBig", bufs=3, space="PSUM") as psBig,
            tc.tile_pool(name="psA", bufs=2, space="PSUM") as psA,
        ):
            # ---- persistent SBUF tensors -------------------------------
            qq_sb = [persist.tile([128, T], DT.bfloat16, tag=f"qq{i}", name=f"qq{i}") for i in range(4)]
            kp_sb = [persist.tile([128, T], DT.bfloat16, tag=f"kp{h}", name=f"kp{h}") for h in range(8)]
            vp_sb = [persist.tile([128, HL * 128], DT.bfloat16, tag=f"vp{i}", name=f"vp{i}") for i in range(NT)]
            ot_sb = [persist.tile([128, T], DT.bfloat16, tag=f"ot{p}", name=f"ot{p}") for p in range(4)]
            wproj_sb = [persist.tile([128, E], DT.bfloat16, tag=f"wp{p}", name=f"wp{p}") for p in range(4)]
            mask_sb = persist.tile([128, 1280], DT.bfloat16, tag="mask", name="mask")
            r_bcast = persist.tile([128, T], DT.float32, tag="r_bcast", name="r_bcast")
            rT_sb = persist.tile([128, 8], DT.float32, tag="rT", name="rT")
            ones16 = persist.tile([128, 1], DT.bfloat16, tag="ones16", name="ones16")
            ones_row = persist.tile([1, 128], DT.float32, tag="ones_row", name="ones_row")
            ones_r = persist.tile([1, 128], DT.float32r, tag="ones_r", name="ones_r")
            one_sc = persist.tile([1, 1], DT.float32, tag="one_sc", name="one_sc")
            r_sb = persist.tile([1, T], DT.float32, tag="r_sb", name="r_sb")
            r_r = persist.tile([1, T], DT.float32r, tag="r_r", name="r_r")
            s_sb = persist.tile([1, T], DT.float32, tag="s_sb", name="s_sb")
            eps_t = persist.tile([1, 1], DT.float32, tag="eps_t", name="eps_t")
            nc.vector.memset(eps_t, float(EPS))
            nc.vector.memset(ones16, 1.0)
            nc.vector.memset(ones_row, 1.0)
            nc.vector.tensor_copy(ones_r, ones_row)
            nc.vector.memset(one_sc, 1.0)

            xt_sb = [persist.tile([128, T], DT.bfloat16, tag=f"xt{i}", name=f"xt{i}") for i in range(8)]
            wqk_sb = [persist.tile([128, 2 * FL], DT.bfloat16, tag=f"wqk{i}", name=f"wqk{i}") for i in range(8)]
            wv_sb = [persist.tile([128, FL], DT.bfloat16, tag=f"wv{i}", name=f"wv{i}") for i in range(8)]
            sq = [persist.tile([128, T], DT.bfloat16, tag=f"sq{i}", name=f"sq{i}") for i in range(8)]

            # ---- DMA in, need-order, HWDGE queues only ------------------
            # sync: xt (gates everything), then wv (V phase), then mask.
            # scalar: wqk (gates qk matmuls), then wproj (c_blocks, last).
            for i in range(8):
                nc.sync.dma_start(out=xt_sb[i], in_=xt_d[i * 128 : (i + 1) * 128, :])
                nc.scalar.dma_start(out=wqk_sb[i], in_=wqk_d[i * 128 : (i + 1) * 128, :])
            for i in range(8):
                nc.sync.dma_start(out=wv_sb[i], in_=wv_d[i * 128 : (i + 1) * 128, :])
            nc.sync.dma_start(out=mask_sb, in_=mask_d[:, :])
            for p in range(4):
                nc.scalar.dma_start(out=wproj_sb[p], in_=wproj_d[p])

            # K pads zeroed on Pool before the first K evictions need them
            for h in range(8):
                pad = slice(0, 64) if h % 2 else slice(64, 128)
                nc.gpsimd.memset(kp_sb[h][pad, :], 0.0)
            # V' block: ones col at 64, zero cols 65:128
            for i in range(NT):
                v3 = vp_sb[i].rearrange("p (h c) -> p h c", h=HL)
                nc.gpsimd.memset(v3[:, :, 64:65], 1.0)
                nc.gpsimd.memset(v3[:, :, 65:128], 0.0)

            # ---- squares on DVE (bf16 out), emitted early ---------------
            for i in range(8):
                nc.vector.tensor_tensor(sq[i], xt_sb[i], xt_sb[i], mybir.AluOpType.mult)

            # ---- phase 1: fi {0,4} both halves, ei-major + ssq ----------
            psQ0 = psBig.tile([128, T], DT.float32, tag="big", name="psQ0")
            psK0 = psBig.tile([128, T], DT.float32, tag="big", name="psK0")
            ssq = [psA.tile([128, 512], DT.float32, tag="ot", name=f"ssq{n}") for n in range(2)]
            for ei in range(8):
                for n in range(2):
                    nc.tensor.matmul(
                        ssq[n][0:1, :], ones16, sq[ei][:, n * 512 : (n + 1) * 512],
                        start=(ei == 0), stop=(ei == 7), skip_group_check=True,
                    )
                for ps, fi in ((psQ0, 0), (psK0, 4)):
                    for n in range(2):
                        nc.tensor.matmul(
                            ps[:, n * 512 : (n + 1) * 512],
                            wqk_sb[ei][:, fi * 128 : (fi + 1) * 128],
                            xt_sb[ei][:, n * 512 : (n + 1) * 512],
                            start=(ei == 0), stop=(ei == 7), skip_group_check=True,
                        )

            # ---- phase 2: r path ---------------------------------------
            for n in range(2):
                half = slice(n * 512, (n + 1) * 512)
                nc.scalar.activation(
                    s_sb[0:1, half], ssq[n][0:1, :], AF.Ln, bias=eps_t, scale=1.0 / E
                )
                nc.scalar.activation(r_sb[0:1, half], s_sb[0:1, half], AF.Exp, scale=-0.5)
            nc.vector.tensor_copy(r_r, r_sb)
            rbp = psBig.tile([128, T], DT.float32, tag="big", name="rbp")
            for n in range(2):
                half = slice(n * 512, (n + 1) * 512)
                nc.tensor.matmul(rbp[:, half], ones_r, r_r[0:1, half], start=True, stop=True)
            nc.vector.tensor_copy(r_bcast, rbp)
            rtp = psA.tile([128, 512], DT.float32, tag="ot", name="rtp")
            for i in range(8):
                nc.tensor.transpose(
                    rtp[:, i : i + 1], r_sb[0:1, i * 128 : (i + 1) * 128], one_sc
                )
            nc.vector.tensor_copy(rT_sb, rtp[:, 0:8])

            # ---- eviction helpers --------------------------------------
            def evict_qk_full(fi, ps):
                """Evict a both-halves [128, T] qk psum tile."""
                if fi < 4:
                    nc.vector.tensor_mul(qq_sb[fi], ps, r_bcast)
                else:
                    for par in range(2):
                        rows = slice(64 * par, 64 * par + 64)
                        nc.vector.tensor_mul(
                            kp_sb[2 * (fi - 4) + par][rows, :], ps[rows, :], r_bcast[rows, :]
                        )

            def evict_qk_half(fi, n, ps):
                half = slice(n * 512, (n + 1) * 512)
                if fi < 4:
                    nc.vector.tensor_mul(qq_sb[fi][:, half], ps[:, 0:512], r_bcast[:, half])
                else:
                    for par in range(2):
                        rows = slice(64 * par, 64 * par + 64)
                        nc.vector.tensor_mul(
                            kp_sb[2 * (fi - 4) + par][rows, half],
                            ps[rows, 0:512], r_bcast[rows, half],
                        )

            # ---- phase 3: evict fi 0/4 ---------------------------------
            evict_qk_full(0, psQ0)
            evict_qk_full(4, psK0)

            # ---- V producer --------------------------------------------
            def v_tile(ti, ps, half):
                hs = slice(half * 512, (half + 1) * 512)
                for ei in range(8):
                    nc.tensor.matmul(
                        ps[:, hs], xt_sb[ei][:, ti * 128 : (ti + 1) * 128], wv_sb[ei],
                        start=(ei == 0), stop=(ei == 7), skip_group_check=True,
                    )
                nc.vector.tensor_scalar_mul(
                    vp_sb[ti].rearrange("p (h c) -> p h c", h=HL)[:, :, 0:64],
                    ps[:, hs].rearrange("p (h c) -> p h c", h=HL),
                    rT_sb[:, ti : ti + 1],
                )

            # ---- phase 4: V0..5 ----------------------------------------
            for g in range(3):
                ps = psBig.tile([128, T], DT.float32, tag="big", name=f"vps{g}")
                for half in range(2):
                    v_tile(2 * g + half, ps, half)

            # ---- phase 5: fi {1,5} both halves -------------------------
            for fi in (1, 5):
                ps = psBig.tile([128, T], DT.float32, tag="big", name=f"qk{fi}")
                for n in range(2):
                    for ei in range(8):
                        nc.tensor.matmul(
                            ps[:, n * 512 : (n + 1) * 512],
                            wqk_sb[ei][:, fi * 128 : (fi + 1) * 128],
                            xt_sb[ei][:, n * 512 : (n + 1) * 512],
                            start=(ei == 0), stop=(ei == 7),
                        )
                evict_qk_full(fi, ps)

            # ---- attention blocks --------------------------------------
            def s_block(b, h):
                """S + EXP for head h, query half b, in key-tile pairs.
                Returns [(p_t, [(j, off, W), ...]), ...]."""
                nj = 4 * b + 4
                qt = qq_sb[h // 2]
                kt = kp_sb[h]
                out = []
                for m in range(nj // 2):
                    j0, j1 = 2 * m, 2 * m + 1
                    k0 = j0 - 4 * b
                    sp = psBig.tile([128, T], DT.float32, tag="big", name="sp")
                    if k0 < 0:  # both full tiles
                        for idx, j in ((0, j0), (1, j1)):
                            nc.tensor.matmul(
                                sp[:, idx * 512 : (idx + 1) * 512],
                                kt[:, j * 128 : (j + 1) * 128],
                                qt[:, b * 512 : (b + 1) * 512],
                                start=True, stop=True, skip_group_check=True,
                            )
                        ents = [(j0, 0, 512), (j1, 512, 512)]
                        ew = 1024
                    else:
                        k1 = k0 + 1
                        W0, W1 = 512 - 128 * k0, 512 - 128 * k1
                        moff = 0 if k0 == 0 else 896
                        ew = W0 + W1
                        nc.vector.tensor_copy(sp[:, 0:ew], mask_sb[:, moff : moff + ew])
                        nc.tensor.matmul(
                            sp[:, 0:W0], kt[:, j0 * 128 : (j0 + 1) * 128],
                            qt[:, b * 512 + 128 * k0 : (b + 1) * 512],
                            start=False, stop=True, skip_group_check=True,
                        )
                        nc.tensor.matmul(
                            sp[:, W0 : W0 + W1], kt[:, j1 * 128 : (j1 + 1) * 128],
                            qt[:, b * 512 + 128 * k1 : (b + 1) * 512],
                            start=False, stop=True, skip_group_check=True,
                        )
                        ents = [(j0, 0, W0), (j1, W0, W1)]
                    p_t = pP.tile([128, T], DT.bfloat16, tag="p_t", name="p_t")
                    nc.scalar.activation(p_t[:, 0:ew], sp[:, 0:ew], AF.Exp, scale=0.125)
                    out.append((p_t, ents))
                return out

            def o_block(b, h, ptiles):
                ot = psA.tile([128, 512], DT.float32, tag="ot", name="ot")
                items = [(j, p_t, off, W) for p_t, ents in ptiles for (j, off, W) in ents]
                for i, (j, p_t, off, W) in enumerate(items):
                    nc.tensor.matmul(
                        ot[:, 512 - W : 512],
                        vp_sb[j][:, h * 128 : (h + 1) * 128],
                        p_t[:, off : off + W],
                        start=(i == 0), stop=(i == len(items) - 1),
                        skip_group_check=True,
                    )
                return ot

            def norm_block(b, h, ot):
                base = 64 * (h % 2)
                trow = work.tile([1, 512], DT.float32r, tag="invr", name="invr")
                nc.scalar.activation(trow, ot[64:65, :], AF.Ln)
                ibp = psBig.tile([128, T], DT.float32, tag="big", name="ibp")
                nc.tensor.matmul(
                    ibp[0:64, 0:512], ones_r[0:1, 0:64], trow, start=True, stop=True
                )
                invb = work.tile([64, 512], DT.float32, tag="invb", name="invb")
                nc.scalar.activation(invb, ibp[0:64, 0:512], AF.Exp, scale=-1.0)
                nc.vector.tensor_mul(
                    ot_sb[h // 2][base : base + 64, b * 512 : (b + 1) * 512],
                    ot[0:64, :], invb,
                )

            # ---- out-proj ----------------------------------------------
            def c_block(ti, chunks=1):
                ps = psBig.tile([128, T], DT.float32, tag="big", name="yps")
                ysb = work.tile([128, T], DT.bfloat16, tag="ysb", name="ysb")
                cw = 512 // chunks
                k = 0
                for n in range(2):
                    for c in range(chunks):
                        cs = slice(n * 512 + c * cw, n * 512 + (c + 1) * cw)
                        for p in range(4):
                            nc.tensor.matmul(
                                ps[:, cs], ot_sb[p][:, ti * 128 : (ti + 1) * 128],
                                wproj_sb[p][:, cs], start=(p == 0), stop=(p == 3),
                            )
                        nc.vector.tensor_copy(ysb[:, cs], ps[:, cs])
                        eng = nc.sync if k % 2 == 0 else nc.scalar
                        k += 1
                        eng.dma_start(
                            out=y_d[ti * 128 : (ti + 1) * 128, cs], in_=ysb[:, cs]
                        )

            # ---- qk filler (single half) -------------------------------
            def filler_qk(fi, n):
                ps = psBig.tile([128, T], DT.float32, tag="big", name=f"fqk{fi}{n}")
                for ei in range(8):
                    nc.tensor.matmul(
                        ps[:, 0:512], wqk_sb[ei][:, fi * 128 : (fi + 1) * 128],
                        xt_sb[ei][:, n * 512 : (n + 1) * 512],
                        start=(ei == 0), stop=(ei == 7),
                    )
                evict_qk_half(fi, n, ps)

            def filler_v(ti):
                ps = psBig.tile([128, T], DT.float32, tag="big", name=f"fv{ti}")
                v_tile(ti, ps, 0)

            # ---- software-pipelined attention + fillers ----------------
            fillers = {
                0: lambda: filler_v(6),
                1: lambda: filler_v(7),
                2: lambda: filler_qk(2, 0),
                3: lambda: filler_qk(6, 0),
                4: lambda: filler_qk(3, 0),
                5: lambda: filler_qk(7, 0),
                6: lambda: filler_qk(2, 1),
                7: lambda: filler_qk(6, 1),
                8: lambda: filler_qk(3, 1),
                9: lambda: filler_qk(7, 1),
                10: lambda: c_block(0),
                11: lambda: c_block(1),
                12: lambda: c_block(2),
                13: lambda: c_block(3),
            }
            seq = [(0, h) for h in range(HL)] + [(1, h) for h in range(HL)]
            pend_o = []
            pend_n = []
            for idx in range(len(seq) + 2):
                # norm first so its Ln isn't queued behind this idx's EXPs
                if idx >= 2 and pend_n:
                    norm_block(*pend_n.pop(0))
                if idx < len(seq):
                    b, h = seq[idx]
                    pend_o.append((b, h, s_block(b, h)))
                if idx in fillers:
                    fillers[idx]()
                if idx >= 1 and pend_o:
                    b, h, ptiles = pend_o.pop(0)
                    pend_n.append((b, h, o_block(b, h, ptiles)))
            for ti in range(4, 8):
                c_block(ti, chunks=2 if ti >= 6 else 1)
    return nc


def _make_masks():
    p = np.arange(128)[:, None]
    w = np.arange(512)[None, :]
    base = np.where(w >= p, 0.0, NEG).astype(np.float32)  # [128, 512]
    m = np.concatenate(
        [base[:, 0:512], base[:, 0:384], base[:, 0:256], base[:, 0:128]], axis=1
    )
    return m.astype(BF16)  # [128, 1280]


def prep_inputs(x, scale, w_qkv, w_proj):
    """Per-core input dict list. Core c: batch c//2, head-group c%2."""
    x = np.asarray(x, np.float32)
    scale = np.asarray(scale, np.float32)
    w_qkv = np.asarray(w_qkv, np.float32)
    w_proj = np.asarray(w_proj, np.float32)
    ws = w_qkv * scale[None, :]  # fold RMSNorm scale into the weights
    mask = _make_masks()
    in_maps = []
    for c in range(N_CORES):
        b, g = c // 2, c % 2
        rows = slice(g * FL, (g + 1) * FL)
        wq = ws[0:E][rows]
        wk = ws[E : 2 * E][rows]
        wv = ws[2 * E : 3 * E][rows]
        wproj_t = np.ascontiguousarray(w_proj[:, rows].T).astype(BF16)  # [FL, E]
        in_maps.append(
            {
                "xt": np.ascontiguousarray(x[b].T).astype(BF16),
                "wqk": np.ascontiguousarray(np.concatenate([wq, wk], 0).T).astype(BF16),
                "wv": np.ascontiguousarray(wv.T).astype(BF16),
                "wproj": wproj_t.reshape(4, 128, E),
                "mask": mask,
            }
        )
    return in_maps


_CACHED_NC = None


def kernel(x, scale, w_qkv, w_proj):
    global _CACHED_NC
    if _CACHED_NC is None:
        _CACHED_NC = build_program()
    in_maps = prep_inputs(x, scale, w_qkv, w_proj)
    res = run_bass_kernel_spmd(_CACHED_NC, in_maps, list(range(N_CORES)))
    out = np.zeros((B, T, E), np.float32)
    for c in range(N_CORES):
        out[c // 2] += res.results[c]["y"].astype(np.float32)
    return out


if __name__ == "__main__":
    rng = np.random.default_rng(0)
    x = rng.standard_normal((B, T, E), dtype=np.float32)
    scale = np.ones(E, np.float32)
    w_qkv = rng.standard_normal((3 * E, E), dtype=np.float32) / 32
    w_proj = rng.standard_normal((E, E), dtype=np.float32) / 32
    y = kernel(x, scale, w_qkv, w_proj)
    print("ran", y.shape, y.dtype, np.abs(y).mean())


# revision 7
# speedup vs baseline: 1.0364x; 1.0364x over previous
"""Causal self-attention (RMSNorm + QKV + causal attention + out-proj) on 8 trn2
NeuronCores.

Sharding: core c handles batch b=c//2 and head-group g=c%2 (Megatron-style TP
over the 16 heads: 8 heads per group). Each core computes a partial output
y_part[b,g] = attn_out(heads of g) @ w_proj[:, g-cols].T ; the host sums the two
TP partials per batch (y shipped bf16, summed fp32 on host).

Restructured pipeline vs the first version: the attention steps (S/EXP/O/norm)
are interleaved with the remaining QKV/V projections and the out-proj c_blocks
so the ACT engine's EXP stream (the attention-phase bottleneck) overlaps the
PE-bound projection work instead of serializing after it. Other changes:
  - sumsq partition-reduce runs on bf16 squares (2.5x faster PE matmuls)
  - S tiles live in 2-bank [128,1024] PSUM pair tiles: one mask-prefill CAST
    and one EXP per pair (48 EXPs, no wasted columns)
  - input DMAs ride the two HWDGE queues (sync: xt+wv+mask, scalar: wqk+wproj)
    in need-order; y out is bf16
"""

import sys

sys.path.insert(0, "/opt/trn_rl_repo")

import numpy as np
import ml_dtypes

import concourse.bass as bass
import concourse.tile as tile
from concourse import mybir
from concourse.bass_utils import run_bass_kernel_spmd

BF16 = ml_dtypes.bfloat16

B, T, E, H = 4, 1024, 1024, 16
D = E // H  # 64
HL = 8  # heads per core (TP degree 2)
FL = HL * D  # 512 local head cols
EPS = 1e-5
N_CORES = 8
NEG = -2400.0  # additive mask; exp(0.125 * -2400) underflows fp32 to 0

AF = mybir.ActivationFunctionType
DT = mybir.dt


# ---------------------------------------------------------------------------
# Walrus in this toolchain rejects instructions whose tail Drain carries more
# than one semaphore wait; split the TileContext exit drain into chained
# single-wait drains.
def _patched_drain_and_barrier(self, tick_clock, wait_clock):
    nc = self.nc
    drain_inst = nc.sync.drain()
    wait_clock.add_sem_waits(
        drain_inst.ins, tile.ScopedClock({None: tick_clock.global_clock})
    )
    mi = drain_inst.ins
    si = mi.sync_info
    if si is not None and len(si.on_wait) > 1:
        waits = list(si.on_wait)
        mi.sync_info = mybir.SyncInfo(on_wait=waits[:1], on_update=list(si.on_update))
        for w in waits[1:]:
            extra = nc.sync.drain().ins
            extra.sync_info = mybir.SyncInfo(on_wait=[w], on_update=[])
    nc.all_engine_barrier()
    assert self.sems is not None
    popped = nc._tile_sem_poison_stack.pop()
    assert popped is self._sem_poison
    nc.clear_and_free_semaphores(list(self.sems.allocated().values()))
    nc.all_engine_barrier()


tile.TileContext._drain_and_barrier = _patched_drain_and_barrier

# The same 1-wait-per-instruction walrus limit applies to every engine's
# instructions. Legalize at BIR-serialization time: hoist all but the last
# wait of a multi-wait instruction onto fresh single-wait Drains inserted
# just before it on the same engine.
_orig_to_json_bytes = bass.Bass.to_json_bytes


def _legalized_to_json_bytes(self):
    import orjson

    j = orjson.loads(_orig_to_json_bytes(self))
    ctr = 0
    for fn in j["functions"]:
        for bb in fn["blocks"]:
            new_insts = []
            for ins in bb["instructions"]:
                si = ins.get("sync_info")
                waits = si.get("on_wait") if si else None
                if waits and len(waits) > 1:
                    for w in waits[:-1]:
                        ctr += 1
                        new_insts.append(
                            {
                                "debug": ins.get("debug"),
                                "engine": ins["engine"],
                                "ins": [],
                                "outs": [],
                                "name": f"I-wf{ctr}",
                                "opcode": "EventSemaphore",
                                "sync_info": {"on_update": [], "on_wait": [w]},
                            }
                        )
                    si["on_wait"] = [waits[-1]]
                new_insts.append(ins)
            bb["instructions"] = new_insts
    return orjson.dumps(j)


bass.Bass.to_json_bytes = _legalized_to_json_bytes


def build_program():
    nc = bass.Bass("TRN2", target_bir_lowering=False, debug=False)

    xt_d = nc.declare_dram_parameter("xt", [E, T], DT.bfloat16, isOutput=False)
    wqk_d = nc.declare_dram_parameter("wqk", [E, 2 * FL], DT.bfloat16, isOutput=False)
    wv_d = nc.declare_dram_parameter("wv", [E, FL], DT.bfloat16, isOutput=False)
    wproj_d = nc.declare_dram_parameter("wproj", [4, 128, E], DT.bfloat16, isOutput=False)
    mask_d = nc.declare_dram_parameter("mask", [128, 1280], DT.bfloat16, isOutput=False)
    y_d = nc.declare_dram_parameter("y", [T, E], DT.bfloat16, isOutput=True)

    NT = T // 128  # 8 tiles of 128

    with tile.TileContext(nc) as tc:
        with (
            tc.tile_pool(name="persist", bufs=1) as persist,
            tc.tile_pool(name="work", bufs=3) as work,
            tc.tile_pool(name="pP", bufs=8) as pP,
            tc.tile_pool(name="psBig", bufs=3, space="PSUM") as psBig,
            tc.tile_pool(name="psA", bufs=2, space="PSUM") as psA,
        ):
            # ---- persistent SBUF tensors -------------------------------
            qq_sb = [persist.tile([128, T], DT.bfloat16, tag=f"qq{i}", name=f"qq{i}") for i in range(4)]
            kp_sb = [persist.tile([128, T], DT.bfloat16, tag=f"kp{h}", name=f"kp{h}") for h in range(8)]
            vp_sb = [persist.tile([128, HL * 128], DT.bfloat16, tag=f"vp{i}", name=f"vp{i}") for i in range(NT)]
            ot_sb = [persist.tile([128, T], DT.bfloat16, tag=f"ot{p}", name=f"ot{p}") for p in range(4)]
            wproj_sb = [persist.tile([128, E], DT.bfloat16, tag=f"wp{p}", name=f"wp{p}") for p in range(4)]
            mask_sb = persist.tile([128, 1280], DT.bfloat16, tag="mask", name="mask")
            r_bcast = persist.tile([128, T], DT.float32, tag="r_bcast", name="r_bcast")
            rT_sb = persist.tile([128, 8], DT.float32, tag="rT", name="rT")
            ones16 = persist.tile([128, 1], DT.bfloat16, tag="ones16", name="ones16")
            ones_row = persist.tile([1, 128], DT.float32, tag="ones_row", name="ones_row")
            ones_r = persist.tile([1, 128], DT.float32r, tag="ones_r", name="ones_r")
            one_sc = persist.tile([1, 1], DT.float32, tag="one_sc", name="one_sc")
            r_sb = persist.tile([1, T], DT.float32, tag="r_sb", name="r_sb")
            r_r = persist.tile([1, T], DT.float32r, tag="r_r", name="r_r")
            s_sb = persist.tile([1, T], DT.float32, tag="s_sb", name="s_sb")
            eps_t = persist.tile([1, 1], DT.float32, tag="eps_t", name="eps_t")
            nc.vector.memset(eps_t, float(EPS))
            nc.vector.memset(ones16, 1.0)
            nc.vector.memset(ones_row, 1.0)
            nc.vector.tensor_copy(ones_r, ones_row)
            nc.vector.memset(one_sc, 1.0)

            xt_sb = [persist.tile([128, T], DT.bfloat16, tag=f"xt{i}", name=f"xt{i}") for i in range(8)]
            wqk_sb = [persist.tile([128, 2 * FL], DT.bfloat16, tag=f"wqk{i}", name=f"wqk{i}") for i in range(8)]
            wv_sb = [persist.tile([128, FL], DT.bfloat16, tag=f"wv{i}", name=f"wv{i}") for i in range(8)]
            sq = [persist.tile([128, T], DT.bfloat16, tag=f"sq{i}", name=f"sq{i}") for i in range(8)]

            # ---- DMA in, need-order, all on the sync HWDGE queue --------
            # (keeps the ACT engine free of DMA-issue work; one queue at
            # full fabric rate, completion order = need order)
            for i in range(8):
                nc.sync.dma_start(out=xt_sb[i], in_=xt_d[i * 128 : (i + 1) * 128, :])
                nc.sync.dma_start(out=wqk_sb[i], in_=wqk_d[i * 128 : (i + 1) * 128, :])
            for i in range(8):
                nc.sync.dma_start(out=wv_sb[i], in_=wv_d[i * 128 : (i + 1) * 128, :])
            nc.sync.dma_start(out=mask_sb, in_=mask_d[:, :])
            for p in range(4):
                nc.sync.dma_start(out=wproj_sb[p], in_=wproj_d[p])

            # K pads zeroed on Pool before the first K evictions need them
            for h in range(8):
                pad = slice(0, 64) if h % 2 else slice(64, 128)
                nc.gpsimd.memset(kp_sb[h][pad, :], 0.0)
            # V' block: ones col at 64, zero cols 65:128
            for i in range(NT):
                v3 = vp_sb[i].rearrange("p (h c) -> p h c", h=HL)
                nc.gpsimd.memset(v3[:, :, 64:65], 1.0)
                nc.gpsimd.memset(v3[:, :, 65:128], 0.0)

            # ---- squares on DVE (bf16 out), emitted early ---------------
            for i in range(8):
                nc.vector.tensor_tensor(sq[i], xt_sb[i], xt_sb[i], mybir.AluOpType.mult)

            # ---- phase 1: fi {0,4} both halves, ei-major + ssq ----------
            psQ0 = psBig.tile([128, T], DT.float32, tag="big", name="psQ0")
            psK0 = psBig.tile([128, T], DT.float32, tag="big", name="psK0")
            ssq = [psA.tile([128, 512], DT.float32, tag="ot", name=f"ssq{n}") for n in range(2)]
            for ei in range(8):
                for n in range(2):
                    nc.tensor.matmul(
                        ssq[n][0:1, :], ones16, sq[ei][:, n * 512 : (n + 1) * 512],
                        start=(ei == 0), stop=(ei == 7), skip_group_check=True,
                    )
                for ps, fi in ((psQ0, 0), (psK0, 4)):
                    for n in range(2):
                        nc.tensor.matmul(
                            ps[:, n * 512 : (n + 1) * 512],
                            wqk_sb[ei][:, fi * 128 : (fi + 1) * 128],
                            xt_sb[ei][:, n * 512 : (n + 1) * 512],
                            start=(ei == 0), stop=(ei == 7), skip_group_check=True,
                        )

            # ---- phase 1b: fi1 matmuls keep the PE busy during r path ---
            psQ1 = psBig.tile([128, T], DT.float32, tag="big", name="psQ1")
            for n in range(2):
                for ei in range(8):
                    nc.tensor.matmul(
                        psQ1[:, n * 512 : (n + 1) * 512],
                        wqk_sb[ei][:, 1 * 128 : 2 * 128],
                        xt_sb[ei][:, n * 512 : (n + 1) * 512],
                        start=(ei == 0), stop=(ei == 7),
                    )

            # ---- phase 2: r path (ACT/DVE; PE only small rbp/rtp) -------
            for n in range(2):
                half = slice(n * 512, (n + 1) * 512)
                nc.scalar.activation(
                    s_sb[0:1, half], ssq[n][0:1, :], AF.Ln, bias=eps_t, scale=1.0 / E
                )
                nc.scalar.activation(r_sb[0:1, half], s_sb[0:1, half], AF.Exp, scale=-0.5)
            nc.vector.tensor_copy(r_r, r_sb)
            for n in range(2):
                half = slice(n * 512, (n + 1) * 512)
                rbp = psA.tile([128, 512], DT.float32, tag="ot", name=f"rbp{n}")
                nc.tensor.matmul(rbp, ones_r, r_r[0:1, half], start=True, stop=True)
                nc.vector.tensor_copy(r_bcast[:, half], rbp)
            rtp = psA.tile([128, 512], DT.float32, tag="ot", name="rtp")
            for i in range(8):
                nc.tensor.transpose(
                    rtp[:, i : i + 1], r_sb[0:1, i * 128 : (i + 1) * 128], one_sc
                )
            nc.vector.tensor_copy(rT_sb, rtp[:, 0:8])

            # ---- eviction helpers --------------------------------------
            def evict_qk_full(fi, ps):
                """Evict a both-halves [128, T] qk psum tile."""
                if fi < 4:
                    nc.vector.tensor_mul(qq_sb[fi], ps, r_bcast)
                else:
                    for par in range(2):
                        rows = slice(64 * par, 64 * par + 64)
                        nc.vector.tensor_mul(
                            kp_sb[2 * (fi - 4) + par][rows, :], ps[rows, :], r_bcast[rows, :]
                        )

            def evict_qk_half(fi, n, ps):
                half = slice(n * 512, (n + 1) * 512)
                if fi < 4:
                    nc.vector.tensor_mul(qq_sb[fi][:, half], ps[:, 0:512], r_bcast[:, half])
                else:
                    for par in range(2):
                        rows = slice(64 * par, 64 * par + 64)
                        nc.vector.tensor_mul(
                            kp_sb[2 * (fi - 4) + par][rows, half],
                            ps[rows, 0:512], r_bcast[rows, half],
                        )

            # ---- phase 3: evicts + fi5 ---------------------------------
            evict_qk_full(0, psQ0)
            psK1 = psBig.tile([128, T], DT.float32, tag="big", name="psK1")
            for n in range(2):
                for ei in range(8):
                    nc.tensor.matmul(
                        psK1[:, n * 512 : (n + 1) * 512],
                        wqk_sb[ei][:, 5 * 128 : 6 * 128],
                        xt_sb[ei][:, n * 512 : (n + 1) * 512],
                        start=(ei == 0), stop=(ei == 7),
                    )
            evict_qk_full(4, psK0)
            evict_qk_full(1, psQ1)
            evict_qk_full(5, psK1)

            # ---- V producer --------------------------------------------
            def v_tile(ti, ps, half):
                hs = slice(half * 512, (half + 1) * 512)
                for ei in range(8):
                    nc.tensor.matmul(
                        ps[:, hs], xt_sb[ei][:, ti * 128 : (ti + 1) * 128], wv_sb[ei],
                        start=(ei == 0), stop=(ei == 7), skip_group_check=True,
                    )
                nc.vector.tensor_scalar_mul(
                    vp_sb[ti].rearrange("p (h c) -> p h c", h=HL)[:, :, 0:64],
                    ps[:, hs].rearrange("p (h c) -> p h c", h=HL),
                    rT_sb[:, ti : ti + 1],
                )

            # ---- phase 4: V0..5 ----------------------------------------
            for g in range(3):
                ps = psBig.tile([128, T], DT.float32, tag="big", name=f"vps{g}")
                for half in range(2):
                    v_tile(2 * g + half, ps, half)

            # ---- attention blocks --------------------------------------
            def s_block(b, h):
                """S + EXP for head h, query half b, in key-tile pairs.
                Returns [(p_t, [(j, off, W), ...]), ...]."""
                nj = 4 * b + 4
                qt = qq_sb[h // 2]
                kt = kp_sb[h]
                out = []
                for m in range(nj // 2):
                    j0, j1 = 2 * m, 2 * m + 1
                    k0 = j0 - 4 * b
                    sp = psBig.tile([128, T], DT.float32, tag="big", name="sp")
                    if k0 < 0:  # both full tiles
                        for idx, j in ((0, j0), (1, j1)):
                            nc.tensor.matmul(
                                sp[:, idx * 512 : (idx + 1) * 512],
                                kt[:, j * 128 : (j + 1) * 128],
                                qt[:, b * 512 : (b + 1) * 512],
                                start=True, stop=True, skip_group_check=True,
                            )
                        ents = [(j0, 0, 512), (j1, 512, 512)]
                        ew = 1024
                    else:
                        k1 = k0 + 1
                        W0, W1 = 512 - 128 * k0, 512 - 128 * k1
                        moff = 0 if k0 == 0 else 896
                        ew = W0 + W1
                        nc.vector.tensor_copy(sp[:, 0:ew], mask_sb[:, moff : moff + ew])
                        nc.tensor.matmul(
                            sp[:, 0:W0], kt[:, j0 * 128 : (j0 + 1) * 128],
                            qt[:, b * 512 + 128 * k0 : (b + 1) * 512],
                            start=False, stop=True, skip_group_check=True,
                        )
                        nc.tensor.matmul(
                            sp[:, W0 : W0 + W1], kt[:, j1 * 128 : (j1 + 1) * 128],
                            qt[:, b * 512 + 128 * k1 : (b + 1) * 512],
                            start=False, stop=True, skip_group_check=True,
                        )
                        ents = [(j0, 0, W0), (j1, W0, W1)]
                    p_t = pP.tile([128, T], DT.bfloat16, tag="p_t", name="p_t")
                    nc.scalar.activation(p_t[:, 0:ew], sp[:, 0:ew], AF.Exp, scale=0.125)
                    out.append((p_t, ents))
                return out

            def o_block(b, h, ptiles):
                ot = psA.tile([128, 512], DT.float32, tag="ot", name="ot")
                items = [(j, p_t, off, W) for p_t, ents in ptiles for (j, off, W) in ents]
                for i, (j, p_t, off, W) in enumerate(items):
                    nc.tensor.matmul(
                        ot[:, 512 - W : 512],
                        vp_sb[j][:, h * 128 : (h + 1) * 128],
                        p_t[:, off : off + W],
                        start=(i == 0), stop=(i == len(items) - 1),
                        skip_group_check=True,
                    )
                return ot

            def norm_block(b, h, ot):
                base = 64 * (h % 2)
                trow = work.tile([1, 512], DT.float32r, tag="invr", name="invr")
                nc.scalar.activation(trow, ot[64:65, :], AF.Ln)
                ibp = psBig.tile([128, T], DT.float32, tag="big", name="ibp")
                nc.tensor.matmul(
                    ibp[0:64, 0:512], ones_r[0:1, 0:64], trow, start=True, stop=True
                )
                invb = work.tile([64, 512], DT.float32, tag="invb", name="invb")
                nc.scalar.activation(invb, ibp[0:64, 0:512], AF.Exp, scale=-1.0)
                nc.vector.tensor_mul(
                    ot_sb[h // 2][base : base + 64, b * 512 : (b + 1) * 512],
                    ot[0:64, :], invb,
                )

            # ---- out-proj ----------------------------------------------
            def c_block(ti, chunks=1):
                ps = psBig.tile([128, T], DT.float32, tag="big", name="yps")
                ysb = work.tile([128, T], DT.bfloat16, tag="ysb", name="ysb")
                cw = 512 // chunks
                k = 0
                for n in range(2):
                    for c in range(chunks):
                        cs = slice(n * 512 + c * cw, n * 512 + (c + 1) * cw)
                        for p in range(4):
                            nc.tensor.matmul(
                                ps[:, cs], ot_sb[p][:, ti * 128 : (ti + 1) * 128],
                                wproj_sb[p][:, cs], start=(p == 0), stop=(p == 3),
                            )
                        nc.vector.tensor_copy(ysb[:, cs], ps[:, cs])
                        k += 1
                        nc.sync.dma_start(
                            out=y_d[ti * 128 : (ti + 1) * 128, cs], in_=ysb[:, cs]
                        )

            # ---- qk filler (single half) -------------------------------
            def filler_qk(fi, n):
                ps = psBig.tile([128, T], DT.float32, tag="big", name=f"fqk{fi}{n}")
                for ei in range(8):
                    nc.tensor.matmul(
                        ps[:, 0:512], wqk_sb[ei][:, fi * 128 : (fi + 1) * 128],
                        xt_sb[ei][:, n * 512 : (n + 1) * 512],
                        start=(ei == 0), stop=(ei == 7),
                    )
                evict_qk_half(fi, n, ps)

            def filler_v(ti):
                ps = psBig.tile([128, T], DT.float32, tag="big", name=f"fv{ti}")
                v_tile(ti, ps, 0)

            # ---- software-pipelined attention + fillers ----------------
            fillers = {
                0: lambda: filler_v(6),
                1: lambda: filler_v(7),
                2: lambda: filler_qk(2, 0),
                3: lambda: filler_qk(6, 0),
                4: lambda: filler_qk(3, 0),
                5: lambda: filler_qk(7, 0),
                6: lambda: filler_qk(2, 1),
                7: lambda: filler_qk(6, 1),
                8: lambda: filler_qk(3, 1),
                9: lambda: filler_qk(7, 1),
                10: lambda: c_block(0),
                11: lambda: c_block(1),
                12: lambda: c_block(2),
                13: lambda: c_block(3),
            }
            seq = [(0, h) for h in range(HL)] + [(1, h) for h in range(HL)]
            pend_o = []
            pend_n = []
            for idx in range(len(seq) + 2):
                # norm first so its Ln isn't queued behind this idx's EXPs
                if idx >= 2 and pend_n:
                    norm_block(*pend_n.pop(0))
                if idx < len(seq):
                    b, h = seq[idx]
                    pend_o.append((b, h, s_block(b, h)))
                if idx in fillers:
                    fillers[idx]()
                if idx >= 1 and pend_o:
                    b, h, ptiles = pend_o.pop(0)
                    pend_n.append((b, h, o_block(b, h, ptiles)))
            for ti in range(4, 8):
                c_block(ti, chunks=2 if ti >= 6 else 1)
    return nc


def _make_masks():
    p = np.arange(128)[:, None]
    w = np.arange(512)[None, :]
    base = np.where(w >= p, 0.0, NEG).astype(np.float32)  # [128, 512]
    m = np.concatenate(
        [base[:, 0:512], base[:, 0:384], base[:, 0:256], base[:, 0:128]], axis=1
    )
    return m.astype(BF16)  # [128, 1280]


def prep_inputs(x, scale, w_qkv, w_proj):
    """Per-core input dict list. Core c: batch c//2, head-group c%2."""
    x = np.asarray(x, np.float32)
    scale = np.asarray(scale, np.float32)
    w_qkv = np.asarray(w_qkv, np.float32)
    w_proj = np.asarray(w_proj, np.float32)
    ws = w_qkv * scale[None, :]  # fold RMSNorm scale into the weights
    mask = _make_masks()
    in_maps = []
    for c in range(N_CORES):
        b, g = c // 2, c % 2
        rows = slice(g * FL, (g + 1) * FL)
        wq = ws[0:E][rows]
        wk = ws[E : 2 * E][rows]
        wv = ws[2 * E : 3 * E][rows]
        wproj_t = np.ascontiguousarray(w_proj[:, rows].T).astype(BF16)  # [FL, E]
        in_maps.append(
            {
                "xt": np.ascontiguousarray(x[b].T).astype(BF16),
                "wqk": np.ascontiguousarray(np.concatenate([wq, wk], 0).T).astype(BF16),
                "wv": np.ascontiguousarray(wv.T).astype(BF16),
                "wproj": wproj_t.reshape(4, 128, E),
                "mask": mask,
            }
        )
    return in_maps


_CACHED_NC = None


def kernel(x, scale, w_qkv, w_proj):
    global _CACHED_NC
    if _CACHED_NC is None:
        _CACHED_NC = build_program()
    in_maps = prep_inputs(x, scale, w_qkv, w_proj)
    res = run_bass_kernel_spmd(_CACHED_NC, in_maps, list(range(N_CORES)))
    out = np.zeros((B, T, E), np.float32)
    for c in range(N_CORES):
        out[c // 2] += res.results[c]["y"].astype(np.float32)
    return out


if __name__ == "__main__":
    rng = np.random.default_rng(0)
    x = rng.standard_normal((B, T, E), dtype=np.float32)
    scale = np.ones(E, np.float32)
    w_qkv = rng.standard_normal((3 * E, E), dtype=np.float32) / 32
    w_proj = rng.standard_normal((E, E), dtype=np.float32) / 32
    y = kernel(x, scale, w_qkv, w_proj)
    print("ran", y.shape, y.dtype, np.abs(y).mean())


# revision 10
# speedup vs baseline: 1.0691x; 1.0315x over previous
"""Causal self-attention (RMSNorm + QKV + causal attention + out-proj) on 8 trn2
NeuronCores.

Sharding: core c handles batch b=c//2 and head-group g=c%2 (Megatron-style TP
over the 16 heads: 8 heads per group). Each core computes a partial output
y_part[b,g] = attn_out(heads of g) @ w_proj[:, g-cols].T ; the host sums the two
TP partials per batch (y shipped bf16, summed fp32 on host).

Restructured pipeline vs the first version: the attention steps (S/EXP/O/norm)
are interleaved with the remaining QKV/V projections and the out-proj c_blocks
so the ACT engine's EXP stream (the attention-phase bottleneck) overlaps the
PE-bound projection work instead of serializing after it. Other changes:
  - sumsq partition-reduce runs on bf16 squares (2.5x faster PE matmuls)
  - S tiles live in 2-bank [128,1024] PSUM pair tiles: one mask-prefill CAST
    and one EXP per pair (48 EXPs, no wasted columns)
  - input DMAs ride the two HWDGE queues (sync: xt+wv+mask, scalar: wqk+wproj)
    in need-order; y out is bf16
"""

import sys

sys.path.insert(0, "/opt/trn_rl_repo")

import numpy as np
import ml_dtypes

import concourse.bass as bass
import concourse.tile as tile
from concourse import mybir
from concourse.bass_utils import run_bass_kernel_spmd

BF16 = ml_dtypes.bfloat16

B, T, E, H = 4, 1024, 1024, 16
D = E // H  # 64
HL = 8  # heads per core (TP degree 2)
FL = HL * D  # 512 local head cols
EPS = 1e-5
N_CORES = 8
NEG = -2400.0  # additive mask; exp(0.125 * -2400) underflows fp32 to 0

AF = mybir.ActivationFunctionType
DT = mybir.dt


# ---------------------------------------------------------------------------
# Walrus in this toolchain rejects instructions whose tail Drain carries more
# than one semaphore wait; split the TileContext exit drain into chained
# single-wait drains.
def _patched_drain_and_barrier(self, tick_clock, wait_clock):
    nc = self.nc
    drain_inst = nc.sync.drain()
    wait_clock.add_sem_waits(
        drain_inst.ins, tile.ScopedClock({None: tick_clock.global_clock})
    )
    mi = drain_inst.ins
    si = mi.sync_info
    if si is not None and len(si.on_wait) > 1:
        waits = list(si.on_wait)
        mi.sync_info = mybir.SyncInfo(on_wait=waits[:1], on_update=list(si.on_update))
        for w in waits[1:]:
            extra = nc.sync.drain().ins
            extra.sync_info = mybir.SyncInfo(on_wait=[w], on_update=[])
    nc.all_engine_barrier()
    assert self.sems is not None
    popped = nc._tile_sem_poison_stack.pop()
    assert popped is self._sem_poison
    nc.clear_and_free_semaphores(list(self.sems.allocated().values()))
    nc.all_engine_barrier()


tile.TileContext._drain_and_barrier = _patched_drain_and_barrier

# The same 1-wait-per-instruction walrus limit applies to every engine's
# instructions. Legalize at BIR-serialization time: hoist all but the last
# wait of a multi-wait instruction onto fresh single-wait Drains inserted
# just before it on the same engine.
_orig_to_json_bytes = bass.Bass.to_json_bytes


def _legalized_to_json_bytes(self):
    import orjson

    j = orjson.loads(_orig_to_json_bytes(self))
    ctr = 0
    for fn in j["functions"]:
        for bb in fn["blocks"]:
            new_insts = []
            for ins in bb["instructions"]:
                si = ins.get("sync_info")
                waits = si.get("on_wait") if si else None
                if waits and len(waits) > 1:
                    for w in waits[:-1]:
                        ctr += 1
                        new_insts.append(
                            {
                                "debug": ins.get("debug"),
                                "engine": ins["engine"],
                                "ins": [],
                                "outs": [],
                                "name": f"I-wf{ctr}",
                                "opcode": "EventSemaphore",
                                "sync_info": {"on_update": [], "on_wait": [w]},
                            }
                        )
                    si["on_wait"] = [waits[-1]]
                new_insts.append(ins)
            bb["instructions"] = new_insts
    return orjson.dumps(j)


bass.Bass.to_json_bytes = _legalized_to_json_bytes


def build_program():
    nc = bass.Bass("TRN2", target_bir_lowering=False, debug=False)

    xt_d = nc.declare_dram_parameter("xt", [E, T], DT.bfloat16, isOutput=False)
    wqk_d = nc.declare_dram_parameter("wqk", [E, 2 * FL], DT.bfloat16, isOutput=False)
    wv_d = nc.declare_dram_parameter("wv", [E, FL], DT.bfloat16, isOutput=False)
    wproj_d = nc.declare_dram_parameter("wproj", [4, 128, E], DT.bfloat16, isOutput=False)
    mask_d = nc.declare_dram_parameter("mask", [128, 1280], DT.bfloat16, isOutput=False)
    y_d = nc.declare_dram_parameter("y", [T, E], DT.bfloat16, isOutput=True)

    NT = T // 128  # 8 tiles of 128

    with tile.TileContext(nc) as tc:
        with (
            tc.tile_pool(name="persist", bufs=1) as persist,
            tc.tile_pool(name="work", bufs=3) as work,
            tc.tile_pool(name="pP", bufs=8) as pP,
            tc.tile_pool(name="psBig", bufs=3, space="PSUM") as psBig,
            tc.tile_pool(name="psA", bufs=2, space="PSUM") as psA,
        ):
            # ---- persistent SBUF tensors -------------------------------
            qq_sb = [persist.tile([128, T], DT.bfloat16, tag=f"qq{i}", name=f"qq{i}") for i in range(4)]
            kp_sb = [persist.tile([128, T], DT.bfloat16, tag=f"kp{h}", name=f"kp{h}") for h in range(8)]
            vp_sb = [persist.tile([128, HL * 128], DT.bfloat16, tag=f"vp{i}", name=f"vp{i}") for i in range(NT)]
            ot_sb = [persist.tile([128, T], DT.bfloat16, tag=f"ot{p}", name=f"ot{p}") for p in range(4)]
            wproj_sb = [persist.tile([128, E], DT.bfloat16, tag=f"wp{p}", name=f"wp{p}") for p in range(4)]
            mask_sb = persist.tile([128, 1280], DT.bfloat16, tag="mask", name="mask")
            r_bcast = persist.tile([128, T], DT.float32, tag="r_bcast", name="r_bcast")
            rT_sb = persist.tile([128, 8], DT.float32, tag="rT", name="rT")
            ones16 = persist.tile([128, 1], DT.bfloat16, tag="ones16", name="ones16")
            ones_row = persist.tile([1, 128], DT.float32, tag="ones_row", name="ones_row")
            ones_r = persist.tile([1, 128], DT.float32r, tag="ones_r", name="ones_r")
            one_sc = persist.tile([1, 1], DT.float32, tag="one_sc", name="one_sc")
            r_sb = persist.tile([1, T], DT.float32, tag="r_sb", name="r_sb")
            r_r = persist.tile([1, T], DT.float32r, tag="r_r", name="r_r")
            s_sb = persist.tile([1, T], DT.float32, tag="s_sb", name="s_sb")
            eps_t = persist.tile([1, 1], DT.float32, tag="eps_t", name="eps_t")
            nc.vector.memset(eps_t, float(EPS))
            nc.vector.memset(ones16, 1.0)
            nc.vector.memset(ones_row, 1.0)
            nc.vector.tensor_copy(ones_r, ones_row)
            nc.vector.memset(one_sc, 1.0)

            xt_sb = [persist.tile([128, T], DT.bfloat16, tag=f"xt{i}", name=f"xt{i}") for i in range(8)]
            wqk_sb = [persist.tile([128, 2 * FL], DT.bfloat16, tag=f"wqk{i}", name=f"wqk{i}") for i in range(8)]
            wv_sb = [persist.tile([128, FL], DT.bfloat16, tag=f"wv{i}", name=f"wv{i}") for i in range(8)]
            sq = [persist.tile([128, T], DT.bfloat16, tag=f"sq{i}", name=f"sq{i}") for i in range(8)]

            # ---- DMA in, need-order, both HWDGE queues ------------------
            # sync: xt then wv (+ y out later); scalar: wqk then mask/wproj
            # (the ACT engine is idle until the first EXPs at ~25us, so the
            # scalar-queue issue cost is free)
            for i in range(8):
                nc.sync.dma_start(out=xt_sb[i], in_=xt_d[i * 128 : (i + 1) * 128, :])
                nc.scalar.dma_start(out=wqk_sb[i], in_=wqk_d[i * 128 : (i + 1) * 128, :])
            for i in range(8):
                nc.sync.dma_start(out=wv_sb[i], in_=wv_d[i * 128 : (i + 1) * 128, :])
            nc.scalar.dma_start(out=mask_sb, in_=mask_d[:, :])
            for p in range(4):
                nc.scalar.dma_start(out=wproj_sb[p], in_=wproj_d[p])

            # K pads zeroed on Pool before the first K evictions need them
            for h in range(8):
                pad = slice(0, 64) if h % 2 else slice(64, 128)
                nc.gpsimd.memset(kp_sb[h][pad, :], 0.0)
            # V' block: ones col at 64, zero cols 65:128
            for i in range(NT):
                v3 = vp_sb[i].rearrange("p (h c) -> p h c", h=HL)
                nc.gpsimd.memset(v3[:, :, 64:65], 1.0)
                nc.gpsimd.memset(v3[:, :, 65:128], 0.0)

            # ---- squares on DVE (bf16 out), emitted early ---------------
            for i in range(8):
                nc.vector.tensor_tensor(sq[i], xt_sb[i], xt_sb[i], mybir.AluOpType.mult)

            # ---- phase 1: fi {0,4} both halves, ei-major + ssq ----------
            psQ0 = psBig.tile([128, T], DT.float32, tag="big", name="psQ0")
            psK0 = psBig.tile([128, T], DT.float32, tag="big", name="psK0")
            ssq = [psA.tile([128, 512], DT.float32, tag="ot", name=f"ssq{n}") for n in range(2)]
            for ei in range(8):
                for n in range(2):
                    nc.tensor.matmul(
                        ssq[n][0:1, :], ones16, sq[ei][:, n * 512 : (n + 1) * 512],
                        start=(ei == 0), stop=(ei == 7), skip_group_check=True,
                    )
                for ps, fi in ((psQ0, 0), (psK0, 4)):
                    for n in range(2):
                        nc.tensor.matmul(
                            ps[:, n * 512 : (n + 1) * 512],
                            wqk_sb[ei][:, fi * 128 : (fi + 1) * 128],
                            xt_sb[ei][:, n * 512 : (n + 1) * 512],
                            start=(ei == 0), stop=(ei == 7), skip_group_check=True,
                        )

            # ---- phase 1b: fi1 matmuls keep the PE busy during r path ---
            psQ1 = psBig.tile([128, T], DT.float32, tag="big", name="psQ1")
            for n in range(2):
                for ei in range(8):
                    nc.tensor.matmul(
                        psQ1[:, n * 512 : (n + 1) * 512],
                        wqk_sb[ei][:, 1 * 128 : 2 * 128],
                        xt_sb[ei][:, n * 512 : (n + 1) * 512],
                        start=(ei == 0), stop=(ei == 7),
                    )

            # ---- phase 2: r path (ACT/DVE; PE only small rbp/rtp) -------
            for n in range(2):
                half = slice(n * 512, (n + 1) * 512)
                nc.scalar.activation(
                    s_sb[0:1, half], ssq[n][0:1, :], AF.Ln, bias=eps_t, scale=1.0 / E
                )
                nc.scalar.activation(r_sb[0:1, half], s_sb[0:1, half], AF.Exp, scale=-0.5)
            nc.vector.tensor_copy(r_r, r_sb)
            for n in range(2):
                half = slice(n * 512, (n + 1) * 512)
                rbp = psA.tile([128, 512], DT.float32, tag="ot", name=f"rbp{n}")
                nc.tensor.matmul(rbp, ones_r, r_r[0:1, half], start=True, stop=True)
                nc.vector.tensor_copy(r_bcast[:, half], rbp)
            rtp = psA.tile([128, 512], DT.float32, tag="ot", name="rtp")
            for i in range(8):
                nc.tensor.transpose(
                    rtp[:, i : i + 1], r_sb[0:1, i * 128 : (i + 1) * 128], one_sc
                )
            nc.vector.tensor_copy(rT_sb, rtp[:, 0:8])

            # ---- eviction helpers --------------------------------------
            def evict_qk_full(fi, ps):
                """Evict a both-halves [128, T] qk psum tile."""
                if fi < 4:
                    nc.vector.tensor_mul(qq_sb[fi], ps, r_bcast)
                else:
                    for par in range(2):
                        rows = slice(64 * par, 64 * par + 64)
                        nc.vector.tensor_mul(
                            kp_sb[2 * (fi - 4) + par][rows, :], ps[rows, :], r_bcast[rows, :]
                        )

            def evict_qk_half(fi, n, ps):
                half = slice(n * 512, (n + 1) * 512)
                if fi < 4:
                    nc.vector.tensor_mul(qq_sb[fi][:, half], ps[:, 0:512], r_bcast[:, half])
                else:
                    for par in range(2):
                        rows = slice(64 * par, 64 * par + 64)
                        nc.vector.tensor_mul(
                            kp_sb[2 * (fi - 4) + par][rows, half],
                            ps[rows, 0:512], r_bcast[rows, half],
                        )

            # ---- phase 3: evicts + fi5 ---------------------------------
            evict_qk_full(0, psQ0)
            psK1 = psBig.tile([128, T], DT.float32, tag="big", name="psK1")
            for n in range(2):
                for ei in range(8):
                    nc.tensor.matmul(
                        psK1[:, n * 512 : (n + 1) * 512],
                        wqk_sb[ei][:, 5 * 128 : 6 * 128],
                        xt_sb[ei][:, n * 512 : (n + 1) * 512],
                        start=(ei == 0), stop=(ei == 7),
                    )
            evict_qk_full(4, psK0)
            evict_qk_full(1, psQ1)
            evict_qk_full(5, psK1)

            # ---- V producer --------------------------------------------
            def v_tile(ti, ps, half):
                hs = slice(half * 512, (half + 1) * 512)
                for ei in range(8):
                    nc.tensor.matmul(
                        ps[:, hs], xt_sb[ei][:, ti * 128 : (ti + 1) * 128], wv_sb[ei],
                        start=(ei == 0), stop=(ei == 7), skip_group_check=True,
                    )
                nc.vector.tensor_scalar_mul(
                    vp_sb[ti].rearrange("p (h c) -> p h c", h=HL)[:, :, 0:64],
                    ps[:, hs].rearrange("p (h c) -> p h c", h=HL),
                    rT_sb[:, ti : ti + 1],
                )

            # ---- phase 4: V0..3 (all b=0 attention needs) ---------------
            for g in range(2):
                ps = psBig.tile([128, T], DT.float32, tag="big", name=f"vps{g}")
                for half in range(2):
                    v_tile(2 * g + half, ps, half)

            # ---- attention blocks --------------------------------------
            def s_block(b, h):
                """S + EXP for head h, query half b, in key-tile pairs.
                Returns [(p_t, [(j, off, W), ...]), ...]."""
                nj = 4 * b + 4
                qt = qq_sb[h // 2]
                kt = kp_sb[h]
                out = []
                for m in range(nj // 2):
                    j0, j1 = 2 * m, 2 * m + 1
                    k0 = j0 - 4 * b
                    sp = psBig.tile([128, T], DT.float32, tag="big", name="sp")
                    if k0 < 0:  # both full tiles
                        for idx, j in ((0, j0), (1, j1)):
                            nc.tensor.matmul(
                                sp[:, idx * 512 : (idx + 1) * 512],
                                kt[:, j * 128 : (j + 1) * 128],
                                qt[:, b * 512 : (b + 1) * 512],
                                start=True, stop=True, skip_group_check=True,
                            )
                        ents = [(j0, 0, 512), (j1, 512, 512)]
                        ew = 1024
                    else:
                        k1 = k0 + 1
                        W0, W1 = 512 - 128 * k0, 512 - 128 * k1
                        moff = 0 if k0 == 0 else 896
                        ew = W0 + W1
                        nc.vector.tensor_copy(sp[:, 0:ew], mask_sb[:, moff : moff + ew])
                        nc.tensor.matmul(
                            sp[:, 0:W0], kt[:, j0 * 128 : (j0 + 1) * 128],
                            qt[:, b * 512 + 128 * k0 : (b + 1) * 512],
                            start=False, stop=True, skip_group_check=True,
                        )
                        nc.tensor.matmul(
                            sp[:, W0 : W0 + W1], kt[:, j1 * 128 : (j1 + 1) * 128],
                            qt[:, b * 512 + 128 * k1 : (b + 1) * 512],
                            start=False, stop=True, skip_group_check=True,
                        )
                        ents = [(j0, 0, W0), (j1, W0, W1)]
                    p_t = pP.tile([128, T], DT.bfloat16, tag="p_t", name="p_t")
                    nc.scalar.activation(p_t[:, 0:ew], sp[:, 0:ew], AF.Exp, scale=0.125)
                    out.append((p_t, ents))
                return out

            def o_block(b, h, ptiles):
                ot = psA.tile([128, 512], DT.float32, tag="ot", name="ot")
                items = [(j, p_t, off, W) for p_t, ents in ptiles for (j, off, W) in ents]
                for i, (j, p_t, off, W) in enumerate(items):
                    nc.tensor.matmul(
                        ot[:, 512 - W : 512],
                        vp_sb[j][:, h * 128 : (h + 1) * 128],
                        p_t[:, off : off + W],
                        start=(i == 0), stop=(i == len(items) - 1),
                        skip_group_check=True,
                    )
                return ot

            def norm_block(b, h, ot):
                base = 64 * (h % 2)
                trow = work.tile([1, 512], DT.float32r, tag="invr", name="invr")
                nc.scalar.activation(trow, ot[64:65, :], AF.Ln)
                ibp = psBig.tile([128, T], DT.float32, tag="big", name="ibp")
                nc.tensor.matmul(
                    ibp[0:64, 0:512], ones_r[0:1, 0:64], trow, start=True, stop=True
                )
                invb = work.tile([64, 512], DT.float32, tag="invb", name="invb")
                nc.scalar.activation(invb, ibp[0:64, 0:512], AF.Exp, scale=-1.0)
                nc.vector.tensor_mul(
                    ot_sb[h // 2][base : base + 64, b * 512 : (b + 1) * 512],
                    ot[0:64, :], invb,
                )

            # ---- out-proj ----------------------------------------------
            def c_block(ti, chunks=1):
                ps = psBig.tile([128, T], DT.float32, tag="big", name="yps")
                ysb = work.tile([128, T], DT.bfloat16, tag="ysb", name="ysb")
                cw = 512 // chunks
                k = 0
                for n in range(2):
                    for c in range(chunks):
                        cs = slice(n * 512 + c * cw, n * 512 + (c + 1) * cw)
                        for p in range(4):
                            nc.tensor.matmul(
                                ps[:, cs], ot_sb[p][:, ti * 128 : (ti + 1) * 128],
                                wproj_sb[p][:, cs], start=(p == 0), stop=(p == 3),
                            )
                        nc.vector.tensor_copy(ysb[:, cs], ps[:, cs])
                        k += 1
                        nc.sync.dma_start(
                            out=y_d[ti * 128 : (ti + 1) * 128, cs], in_=ysb[:, cs]
                        )

            # ---- qk filler (single half) -------------------------------
            def filler_qk(fi, n):
                ps = psBig.tile([128, T], DT.float32, tag="big", name=f"fqk{fi}{n}")
                for ei in range(8):
                    nc.tensor.matmul(
                        ps[:, 0:512], wqk_sb[ei][:, fi * 128 : (fi + 1) * 128],
                        xt_sb[ei][:, n * 512 : (n + 1) * 512],
                        start=(ei == 0), stop=(ei == 7),
                    )
                evict_qk_half(fi, n, ps)

            def filler_v(ti):
                ps = psBig.tile([128, T], DT.float32, tag="big", name=f"fv{ti}")
                v_tile(ti, ps, 0)

            # ---- software-pipelined attention + fillers ----------------
            fillers = {
                0: lambda: filler_qk(2, 0),
                1: lambda: filler_qk(6, 0),
                2: lambda: filler_qk(3, 0),
                3: lambda: filler_qk(7, 0),
                4: lambda: filler_v(4),
                5: lambda: filler_v(5),
                6: lambda: filler_v(6),
                7: lambda: filler_v(7),
                8: lambda: filler_qk(2, 1),
                9: lambda: filler_qk(6, 1),
                10: lambda: filler_qk(3, 1),
                11: lambda: filler_qk(7, 1),
                12: lambda: c_block(0),
                13: lambda: c_block(1),
                14: lambda: c_block(2),
                15: lambda: c_block(3),
            }
            seq = [(0, h) for h in range(HL)] + [(1, h) for h in range(HL)]
            pend_o = []
            pend_n = []
            for idx in range(len(seq) + 2):
                # norm first so its Ln isn't queued behind this idx's EXPs
                if idx >= 2 and pend_n:
                    norm_block(*pend_n.pop(0))
                if idx < len(seq):
                    b, h = seq[idx]
                    pend_o.append((b, h, s_block(b, h)))
                if idx in fillers:
                    fillers[idx]()
                if idx >= 1 and pend_o:
                    b, h, ptiles = pend_o.pop(0)
                    pend_n.append((b, h, o_block(b, h, ptiles)))
            for ti in range(4, 8):
                c_block(ti, chunks=2 if ti >= 6 else 1)
    return nc


def _make_masks():
    p = np.arange(128)[:, None]
    w = np.arange(512)[None, :]
    base = np.where(w >= p, 0.0, NEG).astype(np.float32)  # [128, 512]
    m = np.concatenate(
        [base[:, 0:512], base[:, 0:384], base[:, 0:256], base[:, 0:128]], axis=1
    )
    return m.astype(BF16)  # [128, 1280]


def prep_inputs(x, scale, w_qkv, w_proj):
    """Per-core input dict list. Core c: batch c//2, head-group c%2."""
    x = np.asarray(x, np.float32)
    scale = np.asarray(scale, np.float32)
    w_qkv = np.asarray(w_qkv, np.float32)
    w_proj = np.asarray(w_proj, np.float32)
    ws = w_qkv * scale[None, :]  # fold RMSNorm scale into the weights
    mask = _make_masks()
    in_maps = []
    for c in range(N_CORES):
        b, g = c // 2, c % 2
        rows = slice(g * FL, (g + 1) * FL)
        wq = ws[0:E][rows]
        wk = ws[E : 2 * E][rows]
        wv = ws[2 * E : 3 * E][rows]
        wproj_t = np.ascontiguousarray(w_proj[:, rows].T).astype(BF16)  # [FL, E]
        in_maps.append(
            {
                "xt": np.ascontiguousarray(x[b].T).astype(BF16),
                "wqk": np.ascontiguousarray(np.concatenate([wq, wk], 0).T).astype(BF16),
                "wv": np.ascontiguousarray(wv.T).astype(BF16),
                "wproj": wproj_t.reshape(4, 128, E),
                "mask": mask,
            }
        )
    return in_maps


_CACHED_NC = None


def kernel(x, scale, w_qkv, w_proj):
    global _CACHED_NC
    if _CACHED_NC is None:
        _CACHED_NC = build_program()
    in_maps = prep_inputs(x, scale, w_qkv, w_proj)
    res = run_bass_kernel_spmd(_CACHED_NC, in_maps, list(range(N_CORES)))
    out = np.zeros((B, T, E), np.float32)
    for c in range(N_CORES):
        out[c // 2] += res.results[c]["y"].astype(np.float32)
    return out


if __name__ == "__main__":
    rng = np.random.default_rng(0)
    x = rng.standard_normal((B, T, E), dtype=np.float32)
    scale = np.ones(E, np.float32)
    w_qkv = rng.standard_normal((3 * E, E), dtype=np.float32) / 32
    w_proj = rng.standard_normal((E, E), dtype=np.float32) / 32
    y = kernel(x, scale, w_qkv, w_proj)
    print("ran", y.shape, y.dtype, np.abs(y).mean())
